# revision 54
# baseline (speedup 1.0000x reference)
"""Trainium2 Bass kernel for nn_Grid_fun: out = tile(feat(z), 6) @ a.

Math: z = [x, 1] (N,4); feat = (z otimes z).reshape(N,16); out = tile(feat,6) @ a
    = feat @ a_eff  where a_eff = a.reshape(6,16,3).sum(0)   [16,3]
    => out[n,c] = z[n]^T A_c z[n],  A_c = a_eff[:,c].reshape(4,4)

Device algorithm (per core, data-parallel over N, all-bf16 matmuls):
  Host stages x as Z[3g+j, m] = x[14 m + g, j] (G=14 groups x 3 comps) plus a
  row of ones (43 partition rows, points along the free dim, bf16).
  mm1:  V[127,F] = pv^T @ Z      9 forms per group + 1 shared unit row:
        x0,x1,x2, x0+x1,x0+x2,x1+x2, x0+1,x1+1,x2+1 (via the ones row), 1
  ACT/DVE: R = V^2 elementwise -> bf16 (no bias needed anywhere)
  mm2:  O = ab^T @ R   per group: out_c = sum_s W[c,s] R_s + K'_c
        (universal closed-form weights; constants folded via the unit row)
  Two consecutive tiles pack into one PSUM tile [106,512] at PE base
  partitions 0/64; BOTH mm2s use the identical [127,42] stationary ab, so
  back-to-back mm2s skip the weight reload (measured ~6ns for the second).
  mm1s are emitted in macro-pairs (4 matmuls per pv load). Two DVE copies
  per macro drain the packed halves into two 42-row bf16 SBUF tiles (the
  pad rows 42:64 are never read or written), and per-chunk output DMAs
  alternate the two independent DMA paths.

  DMA (measured on this device): each dma_start's descriptors execute
  serially at ~22.5GB/s, so bandwidth comes from concurrent starts; the
  sync HW-DGE queue (pinned to DMA engine 0) and the gpsimd SW-DGE ring
  are the only two independent paths (scalar shares DMA engine 0 with
  sync). Every input chunk is therefore row-split 15/28 across sync and
  gpsimd so both paths stream all kernel long; consts ride sync; output
  chunks alternate paths with a small final chunk to shorten the tail.
  The last tile is ragged (256 cols, FTOT=8960) so point padding is 0.35%
  instead of 3.2%. mm1 is PSUM-writeback-bound (~427ns/512 cols at 127
  output rows) while mm2 streams at full rate; the matmul, Scalar-square,
  DVE-drain and DMA streams all overlap at ~1.5-1.6us per 1024-col macro.
"""

import sys

if "/opt/trn_rl_repo" not in sys.path:
    sys.path.insert(0, "/opt/trn_rl_repo")

from contextlib import ExitStack

import ml_dtypes
import numpy as np

import concourse.bass as bass
import concourse.mybir as mybir
import concourse.tile as tile
from concourse import bacc
from concourse.bass_utils import run_bass_kernel_spmd

N_CORES = 8
N_POINTS = 1_000_000
N_PER_CORE = N_POINTS // N_CORES  # 125000
G = 14  # points (groups) per column
ZR = 3 * G + 1  # 43 partition rows of Z (incl. the ones row)
VR = 9 * G + 1  # 127 = form rows + shared unit row
T = 512  # matmul free-dim tile
NT = 18  # tiles per core (the last tile is ragged: 256 cols)
NM = NT // 2  # 9 macros (1024-col activation / output super-tiles)
FTOT = 17 * T + 256  # 8960 columns per core (pad waste 0.35%)
NPAD = G * FTOT  # 129024 >= N_PER_CORE
ABW = 3 * G  # ab stationary width (42; both mm2s share the same weights)
OROW = 64 + 3 * G  # 106 rows in the packed output tile
# input DMA chunk column boundaries: tiny first chunk goes via the sync
# HW-DGE queue (lowest latency); the big rest via gpsimd SW-DGE (descriptors
# spread across all 16 physical DMA engines = high bandwidth)
CHB = [0, 1024, 1536, 2560, 3584, 4608, 6144, 7680, 8960]
NCH = len(CHB) - 1
# output DMA chunks (in super-tiles): big early, small last to cut the tail
OCB = [0, 2, 4, 6, 8, 9]
ODR = 3 * G  # 42 rows per packed output tensor (garbage rows dropped)

BF16 = ml_dtypes.bfloat16

_CACHE: dict = {}


def _build_nc():
    nc = bacc.Bacc("TRN2", target_bir_lowering=False)
    f32 = mybir.dt.float32
    bf16 = mybir.dt.bfloat16

    z_d = nc.dram_tensor("z", [ZR, FTOT], bf16, kind="ExternalInput")
    pv_d = nc.dram_tensor("pv", [ZR, VR], bf16, kind="ExternalInput")
    ab_d = nc.dram_tensor("ab", [VR, ABW], bf16, kind="ExternalInput")
    oa_d = nc.dram_tensor("oa", [ODR, NM * T], bf16, kind="ExternalOutput")
    ob_d = nc.dram_tensor("ob", [ODR, NM * T], bf16, kind="ExternalOutput")

    sq = mybir.ActivationFunctionType.Square
    add = mybir.AluOpType.add

    with tile.TileContext(nc) as tc:
        with ExitStack() as ctx:
            cpool = ctx.enter_context(tc.tile_pool(name="consts", bufs=1))
            rpool = ctx.enter_context(tc.tile_pool(name="rt", bufs=2))
            vpool = ctx.enter_context(
                tc.tile_pool(name="vps", bufs=1, space="PSUM")
            )
            opool = ctx.enter_context(
                tc.tile_pool(name="ops", bufs=1, space="PSUM")
            )
            pv = cpool.tile([ZR, VR], bf16)
            ab = cpool.tile([VR, ABW], bf16)
            oa_sb = cpool.tile([ODR, NM * T], bf16)
            ob_sb = cpool.tile([ODR, NM * T], bf16)
            zc = [
                cpool.tile([ZR, CHB[k + 1] - CHB[k]], bf16, name=f"zc{k}")
                for k in range(NCH)
            ]

            # DMA routing: three concurrent paths (each dma_start sustains
            # only ~25-30GB/s). gpsimd SW-DGE carries the early tiles +
            # consts; the sync and scalar HW-DGE queues each stream one late
            # block in parallel.
            # every chunk is row-split ~30/70 across the two independent
            # DMA paths (sync/DMA_0 stream + gpsimd SW-DGE ring) so both
            # run all kernel long: aggregate input bandwidth is the binding
            # constraint, not latency.
            RS = 15
            nc.sync.dma_start(pv[:], pv_d[:, :])
            nc.sync.dma_start(zc[0][0:RS, :], z_d[0:RS, CHB[0] : CHB[1]])
            nc.gpsimd.dma_start(zc[0][RS:ZR, :], z_d[RS:ZR, CHB[0] : CHB[1]])
            nc.sync.dma_start(zc[1][0:RS, :], z_d[0:RS, CHB[1] : CHB[2]])
            nc.gpsimd.dma_start(zc[1][RS:ZR, :], z_d[RS:ZR, CHB[1] : CHB[2]])
            nc.sync.dma_start(ab[:], ab_d[:, :])
            for k in range(2, NCH):
                nc.sync.dma_start(
                    zc[k][0:RS, :], z_d[0:RS, CHB[k] : CHB[k + 1]]
                )
                nc.gpsimd.dma_start(
                    zc[k][RS:ZR, :], z_d[RS:ZR, CHB[k] : CHB[k + 1]]
                )

            # macro pairs: 4x mm1 (one pv weight load), squares, 4x mm2
            # (one ab load) -- halves PE LD_WEIGHTS thrash
            for mp in range(0, NM, 2):
                ms = [m for m in (mp, mp + 1) if m < NM]
                vt, rts, opst = {}, {}, {}
                mw = {m: min(2 * T, FTOT - 2 * m * T) for m in ms}
                for m in ms:
                    vt[m] = vpool.tile([VR, 2 * T], f32, name=f"vps{m % 3}")
                    for h in range(2):
                        c0 = (2 * m + h) * T
                        w = min(T, FTOT - c0)
                        k = next(i for i in range(NCH) if CHB[i + 1] > c0)
                        o0 = c0 - CHB[k]
                        nc.tensor.matmul(
                            vt[m][:, h * T : h * T + w],
                            pv[:],
                            zc[k][:, o0 : o0 + w],
                            start=True,
                            stop=True,
                        )
                for m in ms:
                    rt = rpool.tile([VR, 2 * T], bf16, name=f"rt{m % 2}")
                    rts[m] = rt
                    nc.scalar.activation(
                        rt[:, : mw[m]], vt[m][:, : mw[m]], sq
                    )
                for m in ms:
                    w2 = mw[m] - T
                    ops = opool.tile([OROW, T], f32, name=f"ops{m % 2}")
                    opst[m] = ops
                    nc.tensor.matmul(
                        ops[0 : 3 * G, :], ab[:], rts[m][:, 0:T],
                        start=True, stop=True,
                    )
                    nc.tensor.matmul(
                        ops[64:OROW, 0:w2], ab[:], rts[m][:, T : T + w2],
                        start=True, stop=True,
                    )
                for m in ms:
                    w2 = mw[m] - T
                    nc.vector.tensor_scalar(
                        oa_sb[:, m * T : (m + 1) * T],
                        opst[m][0 : 3 * G, :], 0.0, None, add,
                    )
                    nc.vector.tensor_scalar(
                        ob_sb[:, m * T : m * T + w2],
                        opst[m][64:OROW, 0:w2], 0.0, None, add,
                    )
                    oj = [
                        i for i in range(len(OCB) - 1) if OCB[i + 1] - 1 == m
                    ]
                    if oj:
                        j0 = OCB[oj[0]] * T
                        j1 = min(OCB[oj[0] + 1] * T, (m * T + w2))
                        oeng = nc.sync if oj[0] >= 2 else nc.gpsimd
                        oeng.dma_start(
                            oa_d[:, j0 : OCB[oj[0] + 1] * T],
                            oa_sb[:, j0 : OCB[oj[0] + 1] * T],
                        )
                        nc.gpsimd.dma_start(
                            ob_d[:, j0:j1], ob_sb[:, j0:j1]
                        )
    nc.compile()
    return nc


def _host_tensors(a: np.ndarray):
    """pv / ab from param a [96,3] (exact closed form, fp64)."""
    a_eff = a.astype(np.float64).reshape(6, 16, 3).sum(0)  # [16,3]
    A = a_eff.T.reshape(3, 4, 4)
    As = 0.5 * (A + A.transpose(0, 2, 1))
    Q = As[:, :3, :3]  # [3,3,3] quadratic part
    L = 2.0 * As[:, :3, 3]  # [3,3] linear coefs
    K = As[:, 3, 3]  # [3] constants

    pairs = [(0, 1), (0, 2), (1, 2)]
    W = np.zeros((3, 9))
    for c in range(3):
        for p, (j, k) in enumerate(pairs):
            W[c, 3 + p] = Q[c, j, k]
        for j in range(3):
            W[c, 6 + j] = 0.5 * L[c, j]
            W[c, j] = (
                Q[c, j, j]
                - sum(Q[c, j, k] for k in range(3) if k != j)
                - 0.5 * L[c, j]
            )
    Wones = K - 0.5 * L.sum(axis=1)  # [3]

    pv = np.zeros((ZR, VR), dtype=np.float32)
    ab = np.zeros((VR, ABW), dtype=np.float32)
    for g in range(G):
        for j in range(3):
            pv[3 * g + j, 9 * g + j] = 1.0  # x_j
            pv[3 * g + j, 9 * g + 6 + j] = 1.0  # x_j + 1 ...
            pv[ZR - 1, 9 * g + 6 + j] = 1.0  # ... via the ones row
        for p, (j, k) in enumerate(pairs):
            pv[3 * g + j, 9 * g + 3 + p] = 1.0  # x_j + x_k
            pv[3 * g + k, 9 * g + 3 + p] = 1.0
        for c in range(3):
            for ss in range(9):
                ab[9 * g + ss, 3 * g + c] = W[c, ss]
            ab[VR - 1, 3 * g + c] = Wones[c]
    pv[ZR - 1, VR - 1] = 1.0  # shared unit row
    return pv.astype(BF16), ab.astype(BF16)


def _stage_x(x: np.ndarray, ci: int) -> np.ndarray:
    xs = x[ci * N_PER_CORE : (ci + 1) * N_PER_CORE]
    xp = np.zeros((NPAD, 3), dtype=np.float32)
    xp[:N_PER_CORE] = xs
    z = np.empty((ZR, FTOT), dtype=np.float32)
    z[: ZR - 1] = xp.reshape(FTOT, G, 3).transpose(1, 2, 0).reshape(ZR - 1, FTOT)
    z[ZR - 1] = 1.0
    return z.astype(BF16)


def _decode_o(oa: np.ndarray, ob: np.ndarray) -> np.ndarray:
    """oa/ob [42, 4608] bf16 -> [N_PER_CORE, 3] fp32."""
    tmp = np.stack(
        [oa.astype(np.float32), ob.astype(np.float32)]
    )  # [b, 42, 4608]
    o5 = tmp.reshape(2, G, 3, NM, T)  # [b,g,c,s,w]
    # decode over the full 9216-col grid (m = 1024s+512b+w); cols >= FTOT
    # are garbage and the [:N_PER_CORE] slice drops them
    full = o5.transpose(3, 0, 4, 1, 2).reshape(G * NM * 2 * T, 3)
    return full[:N_PER_CORE]


def kernel(x: np.ndarray, a: np.ndarray) -> np.ndarray:
    x = np.ascontiguousarray(x, dtype=np.float32)
    a = np.ascontiguousarray(a, dtype=np.float32)
    if "nc" not in _CACHE:
        _CACHE["nc"] = _build_nc()
    nc = _CACHE["nc"]

    pv, ab = _host_tensors(a)
    in_maps = []
    for ci in range(N_CORES):
        in_maps.append({"z": _stage_x(x, ci), "pv": pv, "ab": ab})

    res = run_bass_kernel_spmd(nc, in_maps, list(range(N_CORES)))

    out = np.empty((N_POINTS, 3), dtype=np.float32)
    for ci in range(N_CORES):
        out[ci * N_PER_CORE : (ci + 1) * N_PER_CORE] = _decode_o(
            res.results[ci]["oa"], res.results[ci]["ob"]
        )
    return out


# revision 55
# speedup vs baseline: 1.0095x; 1.0095x over previous
"""Trainium2 Bass kernel for nn_Grid_fun: out = tile(feat(z), 6) @ a.

Math: z = [x, 1] (N,4); feat = (z otimes z).reshape(N,16); out = tile(feat,6) @ a
    = feat @ a_eff  where a_eff = a.reshape(6,16,3).sum(0)   [16,3]
    => out[n,c] = z[n]^T A_c z[n],  A_c = a_eff[:,c].reshape(4,4)

Device algorithm (per core, data-parallel over N, all-bf16 matmuls):
  Host stages x as Z[3g+j, m] = x[14 m + g, j] (G=14 groups x 3 comps) plus a
  row of ones (43 partition rows, points along the free dim, bf16).
  mm1:  V[127,F] = pv^T @ Z      9 forms per group + 1 shared unit row:
        x0,x1,x2, x0+x1,x0+x2,x1+x2, x0+1,x1+1,x2+1 (via the ones row), 1
  ACT/DVE: R = V^2 elementwise -> bf16 (no bias needed anywhere)
  mm2:  O = ab^T @ R   per group: out_c = sum_s W[c,s] R_s + K'_c
        (universal closed-form weights; constants folded via the unit row)
  Two consecutive tiles pack into one PSUM tile [106,512] at PE base
  partitions 0/64; BOTH mm2s use the identical [127,42] stationary ab, so
  back-to-back mm2s skip the weight reload (measured ~6ns for the second).
  mm1s are emitted in macro-pairs (4 matmuls per pv load). Two DVE copies
  per macro drain the packed halves into two 42-row bf16 SBUF tiles (the
  pad rows 42:64 are never read or written), and per-chunk output DMAs
  alternate the two independent DMA paths.

  DMA (measured on this device): each dma_start's descriptors execute
  serially at ~22.5GB/s, so bandwidth comes from concurrent starts; the
  sync HW-DGE queue (pinned to DMA engine 0) and the gpsimd SW-DGE ring
  are the only two independent paths (scalar shares DMA engine 0 with
  sync). Every input chunk is therefore row-split 15/28 across sync and
  gpsimd so both paths stream all kernel long; consts ride sync; output
  chunks alternate paths with a small final chunk to shorten the tail.
  The last tile is ragged (256 cols, FTOT=8960) so point padding is 0.35%
  instead of 3.2%. mm1 is PSUM-writeback-bound (~427ns/512 cols at 127
  output rows) while mm2 streams at full rate; the matmul, Scalar-square,
  DVE-drain and DMA streams all overlap at ~1.5-1.6us per 1024-col macro.
"""

import sys

if "/opt/trn_rl_repo" not in sys.path:
    sys.path.insert(0, "/opt/trn_rl_repo")

from contextlib import ExitStack

import ml_dtypes
import numpy as np

import concourse.bass as bass
import concourse.mybir as mybir
import concourse.tile as tile
from concourse import bacc
from concourse.bass_utils import run_bass_kernel_spmd

N_CORES = 8
N_POINTS = 1_000_000
N_PER_CORE = N_POINTS // N_CORES  # 125000
G = 14  # points (groups) per column
ZR = 3 * G + 1  # 43 partition rows of Z (incl. the ones row)
VR = 9 * G + 1  # 127 = form rows + shared unit row
T = 512  # matmul free-dim tile
NT = 18  # tiles per core (the last tile is ragged: 256 cols)
NM = NT // 2  # 9 macros (1024-col activation / output super-tiles)
FTOT = 17 * T + 256  # 8960 columns per core (pad waste 0.35%)
NPAD = G * FTOT  # 129024 >= N_PER_CORE
ABW = 3 * G  # ab stationary width (42; both mm2s share the same weights)
OROW = 64 + 3 * G  # 106 rows in the packed output tile
# input DMA chunk column boundaries: tiny first chunk goes via the sync
# HW-DGE queue (lowest latency); the big rest via gpsimd SW-DGE (descriptors
# spread across all 16 physical DMA engines = high bandwidth)
CHB = [0, 1024, 1536, 2560, 3584, 4608, 6144, 7680, 8960]
NCH = len(CHB) - 1
# output DMA chunks (in super-tiles): big early, small last to cut the tail
OCB = [0, 2, 4, 6, 8, 9]
ODR = 3 * G  # 42 rows per packed output tensor (garbage rows dropped)

BF16 = ml_dtypes.bfloat16

_CACHE: dict = {}


def _build_nc():
    nc = bacc.Bacc("TRN2", target_bir_lowering=False)
    f32 = mybir.dt.float32
    bf16 = mybir.dt.bfloat16

    z_d = nc.dram_tensor("z", [ZR, FTOT], bf16, kind="ExternalInput")
    pv_d = nc.dram_tensor("pv", [ZR, VR], bf16, kind="ExternalInput")
    ab_d = nc.dram_tensor("ab", [VR, ABW], bf16, kind="ExternalInput")
    oa_d = nc.dram_tensor("oa", [ODR, NM * T], bf16, kind="ExternalOutput")
    ob_d = nc.dram_tensor("ob", [ODR, NM * T], bf16, kind="ExternalOutput")

    sq = mybir.ActivationFunctionType.Square
    add = mybir.AluOpType.add

    with tile.TileContext(nc) as tc:
        with ExitStack() as ctx:
            cpool = ctx.enter_context(tc.tile_pool(name="consts", bufs=1))
            rpool = ctx.enter_context(tc.tile_pool(name="rt", bufs=2))
            vpool = ctx.enter_context(
                tc.tile_pool(name="vps", bufs=1, space="PSUM")
            )
            opool = ctx.enter_context(
                tc.tile_pool(name="ops", bufs=1, space="PSUM")
            )
            pv = cpool.tile([ZR, VR], bf16)
            ab = cpool.tile([VR, ABW], bf16)
            oa_sb = cpool.tile([ODR, NM * T], bf16)
            ob_sb = cpool.tile([ODR, NM * T], bf16)
            zc = [
                cpool.tile([ZR, CHB[k + 1] - CHB[k]], bf16, name=f"zc{k}")
                for k in range(NCH)
            ]

            # DMA routing: three concurrent paths (each dma_start sustains
            # only ~25-30GB/s). gpsimd SW-DGE carries the early tiles +
            # consts; the sync and scalar HW-DGE queues each stream one late
            # block in parallel.
            # every chunk is row-split ~30/70 across the two independent
            # DMA paths (sync/DMA_0 stream + gpsimd SW-DGE ring) so both
            # run all kernel long: aggregate input bandwidth is the binding
            # constraint, not latency.
            RS = 15
            nc.sync.dma_start(pv[:], pv_d[:, :])
            nc.sync.dma_start(zc[0][0:RS, :], z_d[0:RS, CHB[0] : CHB[1]])
            nc.gpsimd.dma_start(zc[0][RS:ZR, :], z_d[RS:ZR, CHB[0] : CHB[1]])
            nc.sync.dma_start(zc[1][0:RS, :], z_d[0:RS, CHB[1] : CHB[2]])
            nc.gpsimd.dma_start(zc[1][RS:ZR, :], z_d[RS:ZR, CHB[1] : CHB[2]])
            nc.sync.dma_start(ab[:], ab_d[:, :])
            for k in range(2, NCH):
                nc.sync.dma_start(
                    zc[k][0:RS, :], z_d[0:RS, CHB[k] : CHB[k + 1]]
                )
                nc.gpsimd.dma_start(
                    zc[k][RS:ZR, :], z_d[RS:ZR, CHB[k] : CHB[k + 1]]
                )

            # macro pairs: 4x mm1 (one pv weight load), squares, 4x mm2
            # (one ab load) -- halves PE LD_WEIGHTS thrash
            for mp in range(0, NM, 2):
                ms = [m for m in (mp, mp + 1) if m < NM]
                vt, rts, opst = {}, {}, {}
                mw = {m: min(2 * T, FTOT - 2 * m * T) for m in ms}
                for m in ms:
                    vt[m] = vpool.tile([VR, 2 * T], f32, name=f"vps{m % 2}")
                    for h in range(2):
                        c0 = (2 * m + h) * T
                        w = min(T, FTOT - c0)
                        k = next(i for i in range(NCH) if CHB[i + 1] > c0)
                        o0 = c0 - CHB[k]
                        nc.tensor.matmul(
                            vt[m][:, h * T : h * T + w],
                            pv[:],
                            zc[k][:, o0 : o0 + w],
                            start=True,
                            stop=True,
                        )
                for m in ms:
                    rt = rpool.tile([VR, 2 * T], bf16, name=f"rt{m % 2}")
                    rts[m] = rt
                    nc.scalar.activation(
                        rt[:, : mw[m]], vt[m][:, : mw[m]], sq
                    )
                for m in ms:
                    w2 = mw[m] - T
                    ops = opool.tile([OROW, T], f32, name=f"ops{m % 3}")
                    opst[m] = ops
                    nc.tensor.matmul(
                        ops[0 : 3 * G, :], ab[:], rts[m][:, 0:T],
                        start=True, stop=True,
                    )
                    nc.tensor.matmul(
                        ops[64:OROW, 0:w2], ab[:], rts[m][:, T : T + w2],
                        start=True, stop=True,
                    )
                for m in ms:
                    w2 = mw[m] - T
                    nc.vector.tensor_scalar(
                        oa_sb[:, m * T : (m + 1) * T],
                        opst[m][0 : 3 * G, :], 0.0, None, add,
                    )
                    nc.vector.tensor_scalar(
                        ob_sb[:, m * T : m * T + w2],
                        opst[m][64:OROW, 0:w2], 0.0, None, add,
                    )
                    oj = [
                        i for i in range(len(OCB) - 1) if OCB[i + 1] - 1 == m
                    ]
                    if oj:
                        j0 = OCB[oj[0]] * T
                        j1 = min(OCB[oj[0] + 1] * T, (m * T + w2))
                        oeng = nc.sync if oj[0] >= 2 else nc.gpsimd
                        oeng.dma_start(
                            oa_d[:, j0 : OCB[oj[0] + 1] * T],
                            oa_sb[:, j0 : OCB[oj[0] + 1] * T],
                        )
                        nc.gpsimd.dma_start(
                            ob_d[:, j0:j1], ob_sb[:, j0:j1]
                        )
    nc.compile()
    return nc


def _host_tensors(a: np.ndarray):
    """pv / ab from param a [96,3] (exact closed form, fp64)."""
    a_eff = a.astype(np.float64).reshape(6, 16, 3).sum(0)  # [16,3]
    A = a_eff.T.reshape(3, 4, 4)
    As = 0.5 * (A + A.transpose(0, 2, 1))
    Q = As[:, :3, :3]  # [3,3,3] quadratic part
    L = 2.0 * As[:, :3, 3]  # [3,3] linear coefs
    K = As[:, 3, 3]  # [3] constants

    pairs = [(0, 1), (0, 2), (1, 2)]
    W = np.zeros((3, 9))
    for c in range(3):
        for p, (j, k) in enumerate(pairs):
            W[c, 3 + p] = Q[c, j, k]
        for j in range(3):
            W[c, 6 + j] = 0.5 * L[c, j]
            W[c, j] = (
                Q[c, j, j]
                - sum(Q[c, j, k] for k in range(3) if k != j)
                - 0.5 * L[c, j]
            )
    Wones = K - 0.5 * L.sum(axis=1)  # [3]

    pv = np.zeros((ZR, VR), dtype=np.float32)
    ab = np.zeros((VR, ABW), dtype=np.float32)
    for g in range(G):
        for j in range(3):
            pv[3 * g + j, 9 * g + j] = 1.0  # x_j
            pv[3 * g + j, 9 * g + 6 + j] = 1.0  # x_j + 1 ...
            pv[ZR - 1, 9 * g + 6 + j] = 1.0  # ... via the ones row
        for p, (j, k) in enumerate(pairs):
            pv[3 * g + j, 9 * g + 3 + p] = 1.0  # x_j + x_k
            pv[3 * g + k, 9 * g + 3 + p] = 1.0
        for c in range(3):
            for ss in range(9):
                ab[9 * g + ss, 3 * g + c] = W[c, ss]
            ab[VR - 1, 3 * g + c] = Wones[c]
    pv[ZR - 1, VR - 1] = 1.0  # shared unit row
    return pv.astype(BF16), ab.astype(BF16)


def _stage_x(x: np.ndarray, ci: int) -> np.ndarray:
    xs = x[ci * N_PER_CORE : (ci + 1) * N_PER_CORE]
    xp = np.zeros((NPAD, 3), dtype=np.float32)
    xp[:N_PER_CORE] = xs
    z = np.empty((ZR, FTOT), dtype=np.float32)
    z[: ZR - 1] = xp.reshape(FTOT, G, 3).transpose(1, 2, 0).reshape(ZR - 1, FTOT)
    z[ZR - 1] = 1.0
    return z.astype(BF16)


def _decode_o(oa: np.ndarray, ob: np.ndarray) -> np.ndarray:
    """oa/ob [42, 4608] bf16 -> [N_PER_CORE, 3] fp32."""
    tmp = np.stack(
        [oa.astype(np.float32), ob.astype(np.float32)]
    )  # [b, 42, 4608]
    o5 = tmp.reshape(2, G, 3, NM, T)  # [b,g,c,s,w]
    # decode over the full 9216-col grid (m = 1024s+512b+w); cols >= FTOT
    # are garbage and the [:N_PER_CORE] slice drops them
    full = o5.transpose(3, 0, 4, 1, 2).reshape(G * NM * 2 * T, 3)
    return full[:N_PER_CORE]


def kernel(x: np.ndarray, a: np.ndarray) -> np.ndarray:
    x = np.ascontiguousarray(x, dtype=np.float32)
    a = np.ascontiguousarray(a, dtype=np.float32)
    if "nc" not in _CACHE:
        _CACHE["nc"] = _build_nc()
    nc = _CACHE["nc"]

    pv, ab = _host_tensors(a)
    in_maps = []
    for ci in range(N_CORES):
        in_maps.append({"z": _stage_x(x, ci), "pv": pv, "ab": ab})

    res = run_bass_kernel_spmd(nc, in_maps, list(range(N_CORES)))

    out = np.empty((N_POINTS, 3), dtype=np.float32)
    for ci in range(N_CORES):
        out[ci * N_PER_CORE : (ci + 1) * N_PER_CORE] = _decode_o(
            res.results[ci]["oa"], res.results[ci]["ob"]
        )
    return out


# revision 56
# speedup vs baseline: 1.0194x; 1.0098x over previous
"""Trainium2 Bass kernel for nn_Grid_fun: out = tile(feat(z), 6) @ a.

Math: z = [x, 1] (N,4); feat = (z otimes z).reshape(N,16); out = tile(feat,6) @ a
    = feat @ a_eff  where a_eff = a.reshape(6,16,3).sum(0)   [16,3]
    => out[n,c] = z[n]^T A_c z[n],  A_c = a_eff[:,c].reshape(4,4)

Device algorithm (per core, data-parallel over N, all-bf16 matmuls):
  Host stages x as Z[3g+j, m] = x[14 m + g, j] (G=14 groups x 3 comps) plus a
  row of ones (43 partition rows, points along the free dim, bf16).
  mm1:  V[127,F] = pv^T @ Z      9 forms per group + 1 shared unit row:
        x0,x1,x2, x0+x1,x0+x2,x1+x2, x0+1,x1+1,x2+1 (via the ones row), 1
  ACT/DVE: R = V^2 elementwise -> bf16 (no bias needed anywhere)
  mm2:  O = ab^T @ R   per group: out_c = sum_s W[c,s] R_s + K'_c
        (universal closed-form weights; constants folded via the unit row)
  Two consecutive tiles pack into one PSUM tile [106,512] at PE base
  partitions 0/64; BOTH mm2s use the identical [127,42] stationary ab, so
  back-to-back mm2s skip the weight reload (measured ~6ns for the second).
  mm1s are emitted in macro-pairs (4 matmuls per pv load). Two DVE copies
  per macro drain the packed halves into two 42-row bf16 SBUF tiles (the
  pad rows 42:64 are never read or written), and per-chunk output DMAs
  alternate the two independent DMA paths.

  DMA (measured on this device): each dma_start's descriptors execute
  serially at ~22.5GB/s, so bandwidth comes from concurrent starts; the
  sync HW-DGE queue (pinned to DMA engine 0) and the gpsimd SW-DGE ring
  are the only two independent paths (scalar shares DMA engine 0 with
  sync). Every input chunk is therefore row-split 15/28 across sync and
  gpsimd so both paths stream all kernel long; consts ride sync; output
  chunks alternate paths with a small final chunk to shorten the tail.
  The last tile is ragged (256 cols, FTOT=8960) so point padding is 0.35%
  instead of 3.2%. mm1 is PSUM-writeback-bound (~427ns/512 cols at 127
  output rows) while mm2 streams at full rate; the matmul, Scalar-square,
  DVE-drain and DMA streams all overlap at ~1.5-1.6us per 1024-col macro.
"""

import sys

if "/opt/trn_rl_repo" not in sys.path:
    sys.path.insert(0, "/opt/trn_rl_repo")

from contextlib import ExitStack

import ml_dtypes
import numpy as np

import concourse.bass as bass
import concourse.mybir as mybir
import concourse.tile as tile
from concourse import bacc
from concourse.bass_utils import run_bass_kernel_spmd

N_CORES = 8
N_POINTS = 1_000_000
N_PER_CORE = N_POINTS // N_CORES  # 125000
G = 14  # points (groups) per column
ZR = 3 * G + 1  # 43 partition rows of Z (incl. the ones row)
VR = 9 * G + 1  # 127 = form rows + shared unit row
T = 512  # matmul free-dim tile
NT = 18  # tiles per core (the last tile is ragged: 256 cols)
NM = NT // 2  # 9 macros (1024-col activation / output super-tiles)
FTOT = 17 * T + 256  # 8960 columns per core (pad waste 0.35%)
NPAD = G * FTOT  # 129024 >= N_PER_CORE
ABW = 3 * G  # ab stationary width (42; both mm2s share the same weights)
OROW = 64 + 3 * G  # 106 rows in the packed output tile
# input DMA chunk column boundaries: tiny first chunk goes via the sync
# HW-DGE queue (lowest latency); the big rest via gpsimd SW-DGE (descriptors
# spread across all 16 physical DMA engines = high bandwidth)
CHB = [0, 1024, 1536, 2560, 3584, 4608, 6144, 7680, 8960]
NCH = len(CHB) - 1
# output DMA chunks (in super-tiles): big early, small last to cut the tail
OCB = [0, 2, 4, 6, 8, 9]
ODR = 3 * G  # 42 rows per packed output tensor (garbage rows dropped)

BF16 = ml_dtypes.bfloat16

_CACHE: dict = {}


def _build_nc():
    nc = bacc.Bacc("TRN2", target_bir_lowering=False)
    f32 = mybir.dt.float32
    bf16 = mybir.dt.bfloat16

    z_d = nc.dram_tensor("z", [ZR, FTOT], bf16, kind="ExternalInput")
    pv_d = nc.dram_tensor("pv", [ZR, VR], bf16, kind="ExternalInput")
    ab_d = nc.dram_tensor("ab", [VR, ABW], bf16, kind="ExternalInput")
    oa_d = nc.dram_tensor("oa", [ODR, NM * T], bf16, kind="ExternalOutput")
    ob_d = nc.dram_tensor("ob", [ODR, NM * T], bf16, kind="ExternalOutput")

    sq = mybir.ActivationFunctionType.Square
    add = mybir.AluOpType.add

    with tile.TileContext(nc) as tc:
        with ExitStack() as ctx:
            cpool = ctx.enter_context(tc.tile_pool(name="consts", bufs=1))
            rpool = ctx.enter_context(tc.tile_pool(name="rt", bufs=2))
            vpool = ctx.enter_context(
                tc.tile_pool(name="vps", bufs=1, space="PSUM")
            )
            opool = ctx.enter_context(
                tc.tile_pool(name="ops", bufs=1, space="PSUM")
            )
            pv = cpool.tile([ZR, VR], bf16)
            ab = cpool.tile([VR, ABW], bf16)
            oa_sb = cpool.tile([ODR, NM * T], bf16)
            ob_sb = cpool.tile([ODR, NM * T], bf16)
            zc = [
                cpool.tile([ZR, CHB[k + 1] - CHB[k]], bf16, name=f"zc{k}")
                for k in range(NCH)
            ]

            # DMA routing: three concurrent paths (each dma_start sustains
            # only ~25-30GB/s). gpsimd SW-DGE carries the early tiles +
            # consts; the sync and scalar HW-DGE queues each stream one late
            # block in parallel.
            # every chunk is row-split ~30/70 across the two independent
            # DMA paths (sync/DMA_0 stream + gpsimd SW-DGE ring) so both
            # run all kernel long: aggregate input bandwidth is the binding
            # constraint, not latency.
            RS = 15
            nc.sync.dma_start(pv[:], pv_d[:, :])
            nc.sync.dma_start(zc[0][0:RS, :], z_d[0:RS, CHB[0] : CHB[1]])
            nc.gpsimd.dma_start(zc[0][RS:ZR, :], z_d[RS:ZR, CHB[0] : CHB[1]])
            nc.sync.dma_start(zc[1][0:RS, :], z_d[0:RS, CHB[1] : CHB[2]])
            nc.gpsimd.dma_start(zc[1][RS:ZR, :], z_d[RS:ZR, CHB[1] : CHB[2]])
            nc.sync.dma_start(ab[:], ab_d[:, :])
            for k in range(2, NCH):
                nc.sync.dma_start(
                    zc[k][0:RS, :], z_d[0:RS, CHB[k] : CHB[k + 1]]
                )
                nc.gpsimd.dma_start(
                    zc[k][RS:ZR, :], z_d[RS:ZR, CHB[k] : CHB[k + 1]]
                )

            # macro pairs: 4x mm1 (one pv weight load), squares, 4x mm2
            # (one ab load) -- halves PE LD_WEIGHTS thrash
            for mp in range(0, NM, 2):
                ms = [m for m in (mp, mp + 1) if m < NM]
                vt, rts, opst = {}, {}, {}
                mw = {m: min(2 * T, FTOT - 2 * m * T) for m in ms}
                for m in ms:
                    vt[m] = vpool.tile([VR, 2 * T], f32, name=f"vps{m % 2}")
                    for h in range(2):
                        c0 = (2 * m + h) * T
                        w = min(T, FTOT - c0)
                        k = next(i for i in range(NCH) if CHB[i + 1] > c0)
                        o0 = c0 - CHB[k]
                        nc.tensor.matmul(
                            vt[m][:, h * T : h * T + w],
                            pv[:],
                            zc[k][:, o0 : o0 + w],
                            start=True,
                            stop=True,
                        )
                for m in ms:
                    rt = rpool.tile([VR, 2 * T], bf16, name=f"rt{m % 3}")
                    rts[m] = rt
                    nc.scalar.activation(
                        rt[:, : mw[m]], vt[m][:, : mw[m]], sq
                    )
                for m in ms:
                    w2 = mw[m] - T
                    ops = opool.tile([OROW, T], f32, name=f"ops{m % 3}")
                    opst[m] = ops
                    nc.tensor.matmul(
                        ops[0 : 3 * G, :], ab[:], rts[m][:, 0:T],
                        start=True, stop=True,
                    )
                    nc.tensor.matmul(
                        ops[64:OROW, 0:w2], ab[:], rts[m][:, T : T + w2],
                        start=True, stop=True,
                    )
                for m in ms:
                    w2 = mw[m] - T
                    nc.vector.tensor_scalar(
                        oa_sb[:, m * T : (m + 1) * T],
                        opst[m][0 : 3 * G, :], 0.0, None, add,
                    )
                    nc.vector.tensor_scalar(
                        ob_sb[:, m * T : m * T + w2],
                        opst[m][64:OROW, 0:w2], 0.0, None, add,
                    )
                    oj = [
                        i for i in range(len(OCB) - 1) if OCB[i + 1] - 1 == m
                    ]
                    if oj:
                        j0 = OCB[oj[0]] * T
                        j1 = min(OCB[oj[0] + 1] * T, (m * T + w2))
                        oeng = nc.sync if oj[0] >= 2 else nc.gpsimd
                        oeng.dma_start(
                            oa_d[:, j0 : OCB[oj[0] + 1] * T],
                            oa_sb[:, j0 : OCB[oj[0] + 1] * T],
                        )
                        nc.gpsimd.dma_start(
                            ob_d[:, j0:j1], ob_sb[:, j0:j1]
                        )
    nc.compile()
    return nc


def _host_tensors(a: np.ndarray):
    """pv / ab from param a [96,3] (exact closed form, fp64)."""
    a_eff = a.astype(np.float64).reshape(6, 16, 3).sum(0)  # [16,3]
    A = a_eff.T.reshape(3, 4, 4)
    As = 0.5 * (A + A.transpose(0, 2, 1))
    Q = As[:, :3, :3]  # [3,3,3] quadratic part
    L = 2.0 * As[:, :3, 3]  # [3,3] linear coefs
    K = As[:, 3, 3]  # [3] constants

    pairs = [(0, 1), (0, 2), (1, 2)]
    W = np.zeros((3, 9))
    for c in range(3):
        for p, (j, k) in enumerate(pairs):
            W[c, 3 + p] = Q[c, j, k]
        for j in range(3):
            W[c, 6 + j] = 0.5 * L[c, j]
            W[c, j] = (
                Q[c, j, j]
                - sum(Q[c, j, k] for k in range(3) if k != j)
                - 0.5 * L[c, j]
            )
    Wones = K - 0.5 * L.sum(axis=1)  # [3]

    pv = np.zeros((ZR, VR), dtype=np.float32)
    ab = np.zeros((VR, ABW), dtype=np.float32)
    for g in range(G):
        for j in range(3):
            pv[3 * g + j, 9 * g + j] = 1.0  # x_j
            pv[3 * g + j, 9 * g + 6 + j] = 1.0  # x_j + 1 ...
            pv[ZR - 1, 9 * g + 6 + j] = 1.0  # ... via the ones row
        for p, (j, k) in enumerate(pairs):
            pv[3 * g + j, 9 * g + 3 + p] = 1.0  # x_j + x_k
            pv[3 * g + k, 9 * g + 3 + p] = 1.0
        for c in range(3):
            for ss in range(9):
                ab[9 * g + ss, 3 * g + c] = W[c, ss]
            ab[VR - 1, 3 * g + c] = Wones[c]
    pv[ZR - 1, VR - 1] = 1.0  # shared unit row
    return pv.astype(BF16), ab.astype(BF16)


def _stage_x(x: np.ndarray, ci: int) -> np.ndarray:
    xs = x[ci * N_PER_CORE : (ci + 1) * N_PER_CORE]
    xp = np.zeros((NPAD, 3), dtype=np.float32)
    xp[:N_PER_CORE] = xs
    z = np.empty((ZR, FTOT), dtype=np.float32)
    z[: ZR - 1] = xp.reshape(FTOT, G, 3).transpose(1, 2, 0).reshape(ZR - 1, FTOT)
    z[ZR - 1] = 1.0
    return z.astype(BF16)


def _decode_o(oa: np.ndarray, ob: np.ndarray) -> np.ndarray:
    """oa/ob [42, 4608] bf16 -> [N_PER_CORE, 3] fp32."""
    tmp = np.stack(
        [oa.astype(np.float32), ob.astype(np.float32)]
    )  # [b, 42, 4608]
    o5 = tmp.reshape(2, G, 3, NM, T)  # [b,g,c,s,w]
    # decode over the full 9216-col grid (m = 1024s+512b+w); cols >= FTOT
    # are garbage and the [:N_PER_CORE] slice drops them
    full = o5.transpose(3, 0, 4, 1, 2).reshape(G * NM * 2 * T, 3)
    return full[:N_PER_CORE]


def kernel(x: np.ndarray, a: np.ndarray) -> np.ndarray:
    x = np.ascontiguousarray(x, dtype=np.float32)
    a = np.ascontiguousarray(a, dtype=np.float32)
    if "nc" not in _CACHE:
        _CACHE["nc"] = _build_nc()
    nc = _CACHE["nc"]

    pv, ab = _host_tensors(a)
    in_maps = []
    for ci in range(N_CORES):
        in_maps.append({"z": _stage_x(x, ci), "pv": pv, "ab": ab})

    res = run_bass_kernel_spmd(nc, in_maps, list(range(N_CORES)))

    out = np.empty((N_POINTS, 3), dtype=np.float32)
    for ci in range(N_CORES):
        out[ci * N_PER_CORE : (ci + 1) * N_PER_CORE] = _decode_o(
            res.results[ci]["oa"], res.results[ci]["ob"]
        )
    return out


# revision 57
# speedup vs baseline: 1.0320x; 1.0124x over previous
"""Trainium2 Bass kernel for nn_Grid_fun: out = tile(feat(z), 6) @ a.

Math: z = [x, 1] (N,4); feat = (z otimes z).reshape(N,16); out = tile(feat,6) @ a
    = feat @ a_eff  where a_eff = a.reshape(6,16,3).sum(0)   [16,3]
    => out[n,c] = z[n]^T A_c z[n],  A_c = a_eff[:,c].reshape(4,4)

Device algorithm (per core, data-parallel over N, all-bf16 matmuls):
  Host stages x as Z[3g+j, m] = x[14 m + g, j] (G=14 groups x 3 comps) plus a
  row of ones (43 partition rows, points along the free dim, bf16).
  mm1:  V[127,F] = pv^T @ Z      9 forms per group + 1 shared unit row:
        x0,x1,x2, x0+x1,x0+x2,x1+x2, x0+1,x1+1,x2+1 (via the ones row), 1
  ACT/DVE: R = V^2 elementwise -> bf16 (no bias needed anywhere)
  mm2:  O = ab^T @ R   per group: out_c = sum_s W[c,s] R_s + K'_c
        (universal closed-form weights; constants folded via the unit row)
  Two consecutive tiles pack into one PSUM tile [106,512] at PE base
  partitions 0/64; BOTH mm2s use the identical [127,42] stationary ab, so
  back-to-back mm2s skip the weight reload (measured ~6ns for the second).
  mm1s are emitted in macro-pairs (4 matmuls per pv load). Two DVE copies
  per macro drain the packed halves into two 42-row bf16 SBUF tiles (the
  pad rows 42:64 are never read or written), and per-chunk output DMAs
  alternate the two independent DMA paths.

  DMA (measured on this device): each dma_start's descriptors execute
  serially at ~22.5GB/s, so bandwidth comes from concurrent starts; the
  sync HW-DGE queue (pinned to DMA engine 0) and the gpsimd SW-DGE ring
  are the only two independent paths (scalar shares DMA engine 0 with
  sync). Every input chunk is therefore row-split 15/28 across sync and
  gpsimd so both paths stream all kernel long; consts ride sync; output
  chunks alternate paths with a small final chunk to shorten the tail.
  The last tile is ragged (256 cols, FTOT=8960) so point padding is 0.35%
  instead of 3.2%. mm1 is PSUM-writeback-bound (~427ns/512 cols at 127
  output rows) while mm2 streams at full rate; the matmul, Scalar-square,
  DVE-drain and DMA streams all overlap at ~1.5-1.6us per 1024-col macro.
"""

import sys

if "/opt/trn_rl_repo" not in sys.path:
    sys.path.insert(0, "/opt/trn_rl_repo")

from contextlib import ExitStack

import ml_dtypes
import numpy as np

import concourse.bass as bass
import concourse.mybir as mybir
import concourse.tile as tile
from concourse import bacc
from concourse.bass_utils import run_bass_kernel_spmd

N_CORES = 8
N_POINTS = 1_000_000
N_PER_CORE = N_POINTS // N_CORES  # 125000
G = 14  # points (groups) per column
ZR = 3 * G + 1  # 43 partition rows of Z (incl. the ones row)
VR = 9 * G + 1  # 127 = form rows + shared unit row
T = 512  # matmul free-dim tile
NT = 18  # tiles per core (the last tile is ragged: 256 cols)
NM = NT // 2  # 9 macros (1024-col activation / output super-tiles)
FTOT = 17 * T + 256  # 8960 columns per core (pad waste 0.35%)
NPAD = G * FTOT  # 129024 >= N_PER_CORE
ABW = 3 * G  # ab stationary width (42; both mm2s share the same weights)
OROW = 64 + 3 * G  # 106 rows in the packed output tile
# input DMA chunk column boundaries: tiny first chunk goes via the sync
# HW-DGE queue (lowest latency); the big rest via gpsimd SW-DGE (descriptors
# spread across all 16 physical DMA engines = high bandwidth)
CHB = [0, 1024, 1536, 2560, 3584, 4608, 6144, 7680, 8960]
NCH = len(CHB) - 1
# output DMA chunks (in super-tiles): big early, small last to cut the tail
OCB = [0, 2, 4, 6, 8, 9]
ODR = 3 * G  # 42 rows per packed output tensor (garbage rows dropped)

BF16 = ml_dtypes.bfloat16

_CACHE: dict = {}


def _build_nc():
    nc = bacc.Bacc("TRN2", target_bir_lowering=False)
    f32 = mybir.dt.float32
    bf16 = mybir.dt.bfloat16

    z_d = nc.dram_tensor("z", [ZR, FTOT], bf16, kind="ExternalInput")
    pv_d = nc.dram_tensor("pv", [ZR, VR], bf16, kind="ExternalInput")
    ab_d = nc.dram_tensor("ab", [VR, ABW], bf16, kind="ExternalInput")
    oa_d = nc.dram_tensor("oa", [ODR, NM * T], bf16, kind="ExternalOutput")
    ob_d = nc.dram_tensor("ob", [ODR, NM * T], bf16, kind="ExternalOutput")

    sq = mybir.ActivationFunctionType.Square
    add = mybir.AluOpType.add

    with tile.TileContext(nc) as tc:
        with ExitStack() as ctx:
            cpool = ctx.enter_context(tc.tile_pool(name="consts", bufs=1))
            rpool = ctx.enter_context(tc.tile_pool(name="rt", bufs=2))
            vpool = ctx.enter_context(
                tc.tile_pool(name="vps", bufs=1, space="PSUM")
            )
            opool = ctx.enter_context(
                tc.tile_pool(name="ops", bufs=1, space="PSUM")
            )
            pv = cpool.tile([ZR, VR], bf16)
            ab = cpool.tile([VR, ABW], bf16)
            oa_sb = cpool.tile([ODR, NM * T], bf16)
            ob_sb = cpool.tile([ODR, NM * T], bf16)
            zc = [
                cpool.tile([ZR, CHB[k + 1] - CHB[k]], bf16, name=f"zc{k}")
                for k in range(NCH)
            ]

            # DMA routing: three concurrent paths (each dma_start sustains
            # only ~25-30GB/s). gpsimd SW-DGE carries the early tiles +
            # consts; the sync and scalar HW-DGE queues each stream one late
            # block in parallel.
            # every chunk is row-split ~30/70 across the two independent
            # DMA paths (sync/DMA_0 stream + gpsimd SW-DGE ring) so both
            # run all kernel long: aggregate input bandwidth is the binding
            # constraint, not latency.
            RS = 15
            nc.sync.dma_start(pv[:], pv_d[:, :])
            nc.sync.dma_start(zc[0][0:RS, :], z_d[0:RS, CHB[0] : CHB[1]])
            nc.gpsimd.dma_start(zc[0][RS:ZR, :], z_d[RS:ZR, CHB[0] : CHB[1]])
            nc.sync.dma_start(zc[1][0:RS, :], z_d[0:RS, CHB[1] : CHB[2]])
            nc.gpsimd.dma_start(zc[1][RS:ZR, :], z_d[RS:ZR, CHB[1] : CHB[2]])
            nc.sync.dma_start(ab[:], ab_d[:, :])
            for k in range(2, NCH):
                nc.sync.dma_start(
                    zc[k][0:RS, :], z_d[0:RS, CHB[k] : CHB[k + 1]]
                )
                nc.gpsimd.dma_start(
                    zc[k][RS:ZR, :], z_d[RS:ZR, CHB[k] : CHB[k + 1]]
                )

            # macro pairs: 4x mm1 (one pv weight load), squares, 4x mm2
            # (one ab load) -- halves PE LD_WEIGHTS thrash
            for mp in range(0, NM, 2):
                ms = [m for m in (mp, mp + 1) if m < NM]
                vt, rts, opst = {}, {}, {}
                mw = {m: min(2 * T, FTOT - 2 * m * T) for m in ms}
                for m in ms:
                    vt[m] = vpool.tile([VR, 2 * T], f32, name=f"vps{m % 2}")
                    for h in range(2):
                        c0 = (2 * m + h) * T
                        w = min(T, FTOT - c0)
                        k = next(i for i in range(NCH) if CHB[i + 1] > c0)
                        o0 = c0 - CHB[k]
                        nc.tensor.matmul(
                            vt[m][:, h * T : h * T + w],
                            pv[:],
                            zc[k][:, o0 : o0 + w],
                            start=True,
                            stop=True,
                        )
                for m in ms:
                    rt = rpool.tile([VR, 2 * T], bf16, name=f"rt{m % 3}")
                    rts[m] = rt
                    nc.scalar.activation(
                        rt[:, : mw[m]], vt[m][:, : mw[m]], sq
                    )
                for m in ms:
                    w2 = mw[m] - T
                    ops = opool.tile([OROW, T], f32, name=f"ops{m % 4}")
                    opst[m] = ops
                    nc.tensor.matmul(
                        ops[0 : 3 * G, :], ab[:], rts[m][:, 0:T],
                        start=True, stop=True,
                    )
                    nc.tensor.matmul(
                        ops[64:OROW, 0:w2], ab[:], rts[m][:, T : T + w2],
                        start=True, stop=True,
                    )
                for m in ms:
                    w2 = mw[m] - T
                    nc.vector.tensor_scalar(
                        oa_sb[:, m * T : (m + 1) * T],
                        opst[m][0 : 3 * G, :], 0.0, None, add,
                    )
                    nc.vector.tensor_scalar(
                        ob_sb[:, m * T : m * T + w2],
                        opst[m][64:OROW, 0:w2], 0.0, None, add,
                    )
                    oj = [
                        i for i in range(len(OCB) - 1) if OCB[i + 1] - 1 == m
                    ]
                    if oj:
                        j0 = OCB[oj[0]] * T
                        j1 = min(OCB[oj[0] + 1] * T, (m * T + w2))
                        oeng = nc.sync if oj[0] >= 2 else nc.gpsimd
                        oeng.dma_start(
                            oa_d[:, j0 : OCB[oj[0] + 1] * T],
                            oa_sb[:, j0 : OCB[oj[0] + 1] * T],
                        )
                        nc.gpsimd.dma_start(
                            ob_d[:, j0:j1], ob_sb[:, j0:j1]
                        )
    nc.compile()
    return nc


def _host_tensors(a: np.ndarray):
    """pv / ab from param a [96,3] (exact closed form, fp64)."""
    a_eff = a.astype(np.float64).reshape(6, 16, 3).sum(0)  # [16,3]
    A = a_eff.T.reshape(3, 4, 4)
    As = 0.5 * (A + A.transpose(0, 2, 1))
    Q = As[:, :3, :3]  # [3,3,3] quadratic part
    L = 2.0 * As[:, :3, 3]  # [3,3] linear coefs
    K = As[:, 3, 3]  # [3] constants

    pairs = [(0, 1), (0, 2), (1, 2)]
    W = np.zeros((3, 9))
    for c in range(3):
        for p, (j, k) in enumerate(pairs):
            W[c, 3 + p] = Q[c, j, k]
        for j in range(3):
            W[c, 6 + j] = 0.5 * L[c, j]
            W[c, j] = (
                Q[c, j, j]
                - sum(Q[c, j, k] for k in range(3) if k != j)
                - 0.5 * L[c, j]
            )
    Wones = K - 0.5 * L.sum(axis=1)  # [3]

    pv = np.zeros((ZR, VR), dtype=np.float32)
    ab = np.zeros((VR, ABW), dtype=np.float32)
    for g in range(G):
        for j in range(3):
            pv[3 * g + j, 9 * g + j] = 1.0  # x_j
            pv[3 * g + j, 9 * g + 6 + j] = 1.0  # x_j + 1 ...
            pv[ZR - 1, 9 * g + 6 + j] = 1.0  # ... via the ones row
        for p, (j, k) in enumerate(pairs):
            pv[3 * g + j, 9 * g + 3 + p] = 1.0  # x_j + x_k
            pv[3 * g + k, 9 * g + 3 + p] = 1.0
        for c in range(3):
            for ss in range(9):
                ab[9 * g + ss, 3 * g + c] = W[c, ss]
            ab[VR - 1, 3 * g + c] = Wones[c]
    pv[ZR - 1, VR - 1] = 1.0  # shared unit row
    return pv.astype(BF16), ab.astype(BF16)


def _stage_x(x: np.ndarray, ci: int) -> np.ndarray:
    xs = x[ci * N_PER_CORE : (ci + 1) * N_PER_CORE]
    xp = np.zeros((NPAD, 3), dtype=np.float32)
    xp[:N_PER_CORE] = xs
    z = np.empty((ZR, FTOT), dtype=np.float32)
    z[: ZR - 1] = xp.reshape(FTOT, G, 3).transpose(1, 2, 0).reshape(ZR - 1, FTOT)
    z[ZR - 1] = 1.0
    return z.astype(BF16)


def _decode_o(oa: np.ndarray, ob: np.ndarray) -> np.ndarray:
    """oa/ob [42, 4608] bf16 -> [N_PER_CORE, 3] fp32."""
    tmp = np.stack(
        [oa.astype(np.float32), ob.astype(np.float32)]
    )  # [b, 42, 4608]
    o5 = tmp.reshape(2, G, 3, NM, T)  # [b,g,c,s,w]
    # decode over the full 9216-col grid (m = 1024s+512b+w); cols >= FTOT
    # are garbage and the [:N_PER_CORE] slice drops them
    full = o5.transpose(3, 0, 4, 1, 2).reshape(G * NM * 2 * T, 3)
    return full[:N_PER_CORE]


def kernel(x: np.ndarray, a: np.ndarray) -> np.ndarray:
    x = np.ascontiguousarray(x, dtype=np.float32)
    a = np.ascontiguousarray(a, dtype=np.float32)
    if "nc" not in _CACHE:
        _CACHE["nc"] = _build_nc()
    nc = _CACHE["nc"]

    pv, ab = _host_tensors(a)
    in_maps = []
    for ci in range(N_CORES):
        in_maps.append({"z": _stage_x(x, ci), "pv": pv, "ab": ab})

    res = run_bass_kernel_spmd(nc, in_maps, list(range(N_CORES)))

    out = np.empty((N_POINTS, 3), dtype=np.float32)
    for ci in range(N_CORES):
        out[ci * N_PER_CORE : (ci + 1) * N_PER_CORE] = _decode_o(
            res.results[ci]["oa"], res.results[ci]["ob"]
        )
    return out
